# revision 1
# baseline (speedup 1.0000x reference)
"""Trainium2 Bass kernel for AdaptivePersistenceLandscapeLayer.

Shards the batch (128 samples) across 8 NeuronCores (16 samples each).
Per core:
  - gathers birth/death filtration values from the dtm grid rows via
    gpsimd ap_gather on an fp16 copy of the rows (two calls cover the
    int16 index range), merged by a select
  - computes t_min/t_max per (sample, hom-dim) from the first <=2 pairs
    of that dim (cumsum ranks via tensor_tensor_scan)
  - evaluates tent functions on [t=128partition x 1024pair] tiles
    (4 t-chunks per (sample,dim)); top-1 via tensor_reduce, top-2 via
    exact argmax removal (is_equal + masked re-reduce); tiny per-pair
    perturbation makes tent values unique so duplicate-value argmax
    removal stays exact
  - clamps at 0, adds the adaptive time grid, writes [16, 2, 512, 2]
"""
import contextlib

import numpy as np

import concourse.bass as bass
import concourse.bacc as bacc
import concourse.mybir as mybir
from concourse.tile import TileContext

F32 = mybir.dt.float32
F16 = mybir.dt.float16
I32 = mybir.dt.int32
I16 = mybir.dt.int16
ALU = mybir.AluOpType
AX = mybir.AxisListType

B = 128
N = 65536
P = 1024
PW = 640               # per-dim padded pair-row width (max dim count ~512+8sigma)
T = 512
NCORES = 8
SPC = B // NCORES          # 16 samples per core
NSD = 2 * SPC              # 32 (sample, dim) rows
BIG = 1.0e30
EPS = 6.0e-8


def _install_axon_shim():
    import sys
    import types

    if "antenv.axon_hooks" in sys.modules:
        return
    mod = types.ModuleType("antenv.axon_hooks")
    mod._hook = None
    mod.set_axon_ntff_profile_hook = lambda h: setattr(mod, "_hook", h)
    mod.get_axon_ntff_profile_hook = lambda: mod._hook
    sys.modules["antenv.axon_hooks"] = mod
    try:
        import antenv

        antenv.axon_hooks = mod
    except ImportError:
        pass
    try:
        from trn_agent_boot.trn_boot import _ntff_profile_via_ctypes

        mod._hook = _ntff_profile_via_ctypes("/opt/axon/libaxon_pjrt.so")
    except Exception:
        pass


def build_nc():
    nc = bacc.Bacc("TRN2", target_bir_lowering=False, debug=False)
    dtm = nc.dram_tensor("dtm", [SPC, N], F32, kind="ExternalInput")
    locw = nc.dram_tensor("locw", [128, 320], I32, kind="ExternalInput")
    locc = nc.dram_tensor("locc", [40, 4 * PW], I32, kind="ExternalInput")
    pid_sd_in = nc.dram_tensor("pid_sd", [64, PW], F32, kind="ExternalInput")
    itc_in = nc.dram_tensor("itc", [128, 4], F32, kind="ExternalInput")
    ones_in = nc.dram_tensor("ones_c", [1, 128], F32, kind="ExternalInput")
    ident_in = nc.dram_tensor("ident", [128, 128], F32, kind="ExternalInput")
    out = nc.dram_tensor("out", [SPC, 2, T, 2], F32, kind="ExternalOutput")

    with TileContext(nc) as tc:
        with contextlib.ExitStack() as _st:
            gp0 = _st.enter_context(tc.tile_pool(name="gp0", bufs=1))
            # ---------------- gather phase ----------------
            # pair index loc>>1 spans [0, 32767] == int16 range: one full-row
            # fp16 table per 8 samples, single gather per round
            bdp = gp0.tile([40, 8 * PW], F16, tag="bdp")
            locw_t = gp0.tile([128, 320], I32, tag="locw")
            nc.sync.dma_start(locw_t[:], locw[:])
            fli = gp0.tile([128, 320], I32, tag="fli")
            nc.vector.tensor_scalar(out=fli[:], in0=locw_t[:], scalar1=1,
                                    scalar2=None, op0=ALU.arith_shift_right)
            idx16 = gp0.tile([128, 320], I16, tag="idx16")
            nc.vector.tensor_copy(idx16[:], fli[:])

            with tc.tile_pool(name="dtp", bufs=1) as dtp:
                dt16 = dtp.tile([128, N], F16, tag="dt16")
                ga = dtp.tile([128, 8 * PW], F16, tag="ga")
                nc.vector.memset(dt16[:, :N // 2], 0.0)
                nc.vector.memset(dt16[:, N // 2:], 0.0)
                _gsid, _ = nc.enter_named_scope("gatherphase", False)
                for rnd in range(2):
                    for c in range(8):
                        nc.gpsimd.dma_start(
                            dt16[16 * c:16 * c + 1, :],
                            dtm[8 * rnd + c:8 * rnd + c + 1, :])
                    nc.gpsimd.ap_gather(
                        ga[:], dt16[:], idx16[:, 160 * rnd:160 * (rnd + 1)],
                        channels=128, num_elems=N // 2, d=2, num_idxs=4 * PW)
                    for c in range(8):
                        nc.sync.dma_start(
                            bdp[32 * rnd + c:32 * rnd + c + 1, :],
                            ga[16 * c:16 * c + 1, :])

            nc.leave_named_scope("gatherphase", _gsid, False)
            gp = _st.enter_context(tc.tile_pool(name="gp", bufs=1))
            # pre-round select masks (rows 32r+si used; others ignored)
            locc_t = gp.tile([40, 4 * PW], I32, tag="locc")
            nc.sync.dma_start(locc_t[:], locc[:])
            pari = gp.tile([40, 4 * PW], I32, tag="pari")
            nc.vector.tensor_scalar(out=pari[:], in0=locc_t[:], scalar1=1,
                                    scalar2=None, op0=ALU.bitwise_and)
            par = gp.tile([40, 4 * PW], F32, tag="flh")
            nc.vector.tensor_copy(par[:], pari[:])
            parn = gp.tile([40, 4 * PW], F32, tag="hsel2")
            nc.vector.tensor_scalar(out=parn[:], in0=par[:], scalar1=-1.0,
                                    scalar2=1.0, op0=ALU.mult, op1=ALU.add)
            id_t = gp.tile([128, 128], F32, tag="ident")
            nc.sync.dma_start(id_t[:], ident_in[:])
            itc_t = gp.tile([128, 4], F32, tag="itc")
            nc.sync.dma_start(itc_t[:], itc_in[:])
            pid_sd = gp.tile([64, PW], F32, tag="pid_sd")
            nc.sync.dma_start(pid_sd[:], pid_sd_in[:])

            bdf = gp.tile([40, 4 * PW], F32, tag="bdf")
            bdf2 = gp.tile([40, 4 * PW], F32, tag="pari")
            birth_sd = gp.tile([64, PW], F32, tag="birth_sd")
            death_sd = gp.tile([64, PW], F32, tag="death_sd")
            nb_sd = gp.tile([64, PW], F32, tag="nb_sd")
            tmin = gp.tile([64, 1], F32, tag="tmin")
            tmax = gp.tile([64, 1], F32, tag="tmax")
            delta = gp.tile([64, 1], F32, tag="delta")
            tm2 = gp.tile([64, 2], F32, tag="tm2")
            tseq = gp.tile([128, 64 * 4], F32, tag="tseq")
            lam = gp.tile([128, 64 * 4 * 2], F32, tag="lam")

            nc.vector.memset(lam[:], 0.0)
            nc.vector.memset(tseq[:], 0.0)
            _tsid, _ = nc.enter_named_scope("tentphase", False)
            with tc.tile_pool(name="wp", bufs=2) as wp, \
                 tc.tile_pool(name="pp0", bufs=1, space="PSUM") as pp0, \
                 tc.tile_pool(name="pp", bufs=1, space="PSUM") as pp:
                for rnd in range(2):
                    q = 32 * rnd
                    sl = slice(q, q + 8)          # merge rows
                    rl = slice(q, q + 16)         # sd rows
                    # merge this round's 8 samples (parity select within pair)
                    nc.vector.tensor_copy(bdf[sl, :], bdp[sl, 0::2])
                    nc.vector.tensor_copy(bdf2[sl, :], bdp[sl, 1::2])
                    nc.vector.tensor_tensor(out=bdf[sl, :], in0=bdf[sl, :],
                                            in1=parn[sl, :], op=ALU.mult)
                    nc.vector.tensor_tensor(out=bdf2[sl, :], in0=bdf2[sl, :],
                                            in1=par[sl, :], op=ALU.mult)
                    nc.vector.tensor_tensor(out=bdf[sl, :], in0=bdf[sl, :],
                                            in1=bdf2[sl, :], op=ALU.add)
                    # sd rows: q + d*8 + si  <- bdf rows q+si, col-block d
                    for d in range(2):
                        r0 = q + d * 8
                        nc.sync.dma_start(birth_sd[r0:r0 + 8, :],
                                          bdf[sl, d * PW:(d + 1) * PW])
                        nc.sync.dma_start(death_sd[r0:r0 + 8, :],
                                          bdf[sl, (2 + d) * PW:(3 + d) * PW])
                    nc.vector.tensor_reduce(out=tmin[rl, :],
                                            in_=birth_sd[rl, 0:2],
                                            axis=AX.X, op=ALU.min)
                    nc.vector.tensor_reduce(out=tmax[rl, :],
                                            in_=death_sd[rl, 0:2],
                                            axis=AX.X, op=ALU.max)
                    nc.vector.tensor_tensor(out=delta[rl, :], in0=tmax[rl, :],
                                            in1=tmin[rl, :], op=ALU.subtract)
                    nc.vector.tensor_scalar(out=delta[rl, :], in0=delta[rl, :],
                                            scalar1=1.0 / 511.0, scalar2=None,
                                            op0=ALU.mult)
                    nc.vector.tensor_copy(tm2[rl, 0:1], tmin[rl, :])
                    nc.vector.tensor_copy(tm2[rl, 1:2], delta[rl, :])
                    nc.vector.tensor_tensor(out=nb_sd[rl, :], in0=pid_sd[rl, :],
                                            in1=birth_sd[rl, :], op=ALU.subtract)
                    nc.vector.tensor_tensor(out=death_sd[rl, :],
                                            in0=death_sd[rl, :],
                                            in1=pid_sd[rl, :], op=ALU.add)
                    # tseq for this round's 16 sd rows
                    tm2T_p = pp0.tile([2, 16], F32, tag="tm2T")
                    nc.tensor.transpose(out=tm2T_p[:], in_=tm2[rl, :],
                                        identity=id_t[q:q + 16, q:q + 16])
                    tm2T = wp.tile([2, 16], F32, tag="tm2Ts")
                    nc.vector.tensor_copy(tm2T[:], tm2T_p[:])
                    tminb_p = pp0.tile([128, 16], F32, tag="tminb")
                    deltab_p = pp0.tile([128, 16], F32, tag="deltab")
                    nc.tensor.matmul(out=tminb_p[:],
                                     lhsT=id_t[:2, 0:1].to_broadcast([2, 128]),
                                     rhs=tm2T[:], start=True, stop=True)
                    nc.tensor.matmul(out=deltab_p[:],
                                     lhsT=id_t[:2, 1:2].to_broadcast([2, 128]),
                                     rhs=tm2T[:], start=True, stop=True)
                    itc_v = itc_t[:].unsqueeze(1).to_broadcast([128, 16, 4])
                    db_v = deltab_p[:].unsqueeze(2).to_broadcast([128, 16, 4])
                    tb_v = tminb_p[:].unsqueeze(2).to_broadcast([128, 16, 4])
                    tseq3 = tseq[:, q * 4:(q + 16) * 4].rearrange(
                        "p (a b) -> p a b", b=4)
                    nc.vector.tensor_tensor(out=tseq3, in0=itc_v, in1=db_v,
                                            op=ALU.mult)
                    nc.vector.tensor_tensor(out=tseq3, in0=tseq3, in1=tb_v,
                                            op=ALU.add)
                    # tents
                    for lrow in range(16):
                        row = q + lrow
                        nbb = pp.tile([128, PW], F32, tag="nbb")
                        dbb = pp.tile([128, PW], F32, tag="dbb")
                        sel_l = id_t[q:q + 16, row:row + 1].to_broadcast([16, 128])
                        for h0, h1 in ((0, 512), (512, PW)):
                            nc.tensor.matmul(
                                out=nbb[:, h0:h1], lhsT=sel_l,
                                rhs=nb_sd[rl, h0:h1], start=True, stop=True)
                            nc.tensor.matmul(
                                out=dbb[:, h0:h1], lhsT=sel_l,
                                rhs=death_sd[rl, h0:h1], start=True, stop=True)
                        for cp in range(2):
                            tent2 = wp.tile([128, 2 * PW], F32, tag="tent")
                            eq2 = wp.tile([128, 2 * PW], F32, tag="eq")
                            for cc in range(2):
                                c = 2 * cp + cc
                                tcol = tseq[:, row * 4 + c:row * 4 + c + 1]
                                ctv = eq2[:, cc * PW:(cc + 1) * PW]
                                nc.vector.tensor_scalar(
                                    out=ctv, in0=dbb[:], scalar1=tcol,
                                    scalar2=None, op0=ALU.subtract)
                                nc.vector.scalar_tensor_tensor(
                                    out=tent2[:, cc * PW:(cc + 1) * PW],
                                    in0=nbb[:], scalar=tcol, in1=ctv,
                                    op0=ALU.add, op1=ALU.min)
                            col0 = (row * 4 + 2 * cp) * 2
                            nc.vector.tensor_reduce(
                                out=lam[:, col0:col0 + 4:2],
                                in_=tent2[:].rearrange("p (a b) -> p a b", b=PW),
                                axis=AX.X, op=ALU.max)
                            for cc in range(2):
                                c = 2 * cp + cc
                                col = (row * 4 + c) * 2
                                nc.vector.tensor_scalar(
                                    out=eq2[:, cc * PW:(cc + 1) * PW],
                                    in0=tent2[:, cc * PW:(cc + 1) * PW],
                                    scalar1=lam[:, col:col + 1],
                                    scalar2=None, op0=ALU.is_equal)
                                nc.vector.scalar_tensor_tensor(
                                    out=eq2[:, cc * PW:(cc + 1) * PW],
                                    in0=eq2[:, cc * PW:(cc + 1) * PW], scalar=-BIG,
                                    in1=tent2[:, cc * PW:(cc + 1) * PW],
                                    op0=ALU.mult, op1=ALU.add)
                            nc.vector.tensor_reduce(
                                out=lam[:, col0 + 1:col0 + 5:2],
                                in_=eq2[:].rearrange("p (a b) -> p a b", b=PW),
                                axis=AX.X, op=ALU.max)
            nc.leave_named_scope("tentphase", _tsid, False)

            # clamp and add tseq
            nc.vector.tensor_scalar(out=lam[:], in0=lam[:], scalar1=0.0,
                                    scalar2=None, op0=ALU.max)
            tseq_r = tseq[:].unsqueeze(2).to_broadcast([128, 64 * 4, 2])
            lam3 = lam[:].rearrange("p (a b) -> p a b", b=2)
            nc.vector.tensor_tensor(out=lam3, in0=lam3, in1=tseq_r, op=ALU.add)

            # ---------------- output ----------------
            for rnd in range(2):
                for d in range(2):
                    for c in range(4):
                        dst = out.ap().rearrange("s d (c t) k -> s d c t k", c=4)[
                            8 * rnd:8 * rnd + 8, d, c, :, :].rearrange(
                            "s t k -> t s k")
                        src = lam[:].rearrange("t (b s c k) -> t b s c k",
                                               b=8, s=8, c=4)[:, 4 * rnd + d, :, c, :]
                        nc.sync.dma_start(dst, src)
    nc.compile()
    return nc


_NC_CACHE = None
_LAST_IN_MAPS = None


def kernel(dtm_val, birth_loc, death_loc, ph_dim):
    global _NC_CACHE
    _install_axon_shim()
    from concourse.bass_utils import run_bass_kernel_spmd

    dtm_val = np.ascontiguousarray(np.asarray(dtm_val, dtype=np.float32))
    birth_loc = np.asarray(birth_loc, dtype=np.int32)
    death_loc = np.asarray(death_loc, dtype=np.int32)
    ph_dim = np.asarray(ph_dim, dtype=np.int32)

    if _NC_CACHE is None:
        _NC_CACHE = build_nc()
    nc = _NC_CACHE

    itc = np.zeros((128, 4), np.float32)
    for c in range(4):
        itc[:, c] = 128 * c + np.arange(128)
    ones_c = np.ones((1, 128), np.float32)
    ident = np.eye(128, dtype=np.float32)

    in_maps = []
    for i in range(NCORES):
        s0 = i * SPC
        locc = np.zeros((40, 4 * PW), np.int32)
        pid_sd = np.full((64, PW), -BIG, np.float32)
        for si in range(SPC):
            rnd, sloc = si // 8, si % 8
            ph = ph_dim[s0 + si]
            for d in range(2):
                pos = np.where(ph == d)[0]
                assert len(pos) <= PW, f"dim count {len(pos)} exceeds PW={PW}"
                n = len(pos)
                locc[32 * rnd + sloc, d * PW:d * PW + n] = birth_loc[s0 + si, pos]
                locc[32 * rnd + sloc, (2 + d) * PW:(2 + d) * PW + n] = (
                    death_loc[s0 + si, pos])
                pid_sd[32 * rnd + d * 8 + sloc, :n] = pos.astype(np.float32) * EPS
        locw = np.zeros((128, 320), np.int32)
        for rnd in range(2):
            for c in range(8):
                lst = locc[32 * rnd + c]
                locw[16 * c:16 * (c + 1), 160 * rnd:160 * (rnd + 1)] = (
                    lst.reshape(160, 16).T)
        in_maps.append({
            "dtm": dtm_val[s0:s0 + SPC],
            "locw": locw,
            "locc": locc,
            "pid_sd": pid_sd,
            "itc": itc,
            "ones_c": ones_c,
            "ident": ident,
        })

    global _LAST_IN_MAPS
    _LAST_IN_MAPS = in_maps
    res = run_bass_kernel_spmd(nc, in_maps, core_ids=list(range(NCORES)))
    outs = [r["out"] for r in res.results]
    return np.concatenate(outs, axis=0).astype(np.float32)



# revision 2
# speedup vs baseline: 1.1042x; 1.1042x over previous
"""Trainium2 Bass kernel v2 for AdaptivePersistenceLandscapeLayer.

Strategy (per core, 16 samples):
  1. fp16 tables: dtm row of sample (8r+k) -> SBUF partition 16k+r (casting
     DMAs via gpsimd). Two ap_gather rounds (shared per-DSP-core index lists)
     fetch birth/death values for the locc slot layout.
  2. Merge parity (e/o) with host masks; pack words [b|h] (h = d-b) as f32;
     dead pairs (h <= 0, i.e. death <= birth -- tent never positive) get
     negative words.
  3. gpsimd sparse_gather compacts away dead pairs per (sample, dim):
     576 padded slots -> 320 live slots (actual live max 292).
  4. Tent phase per (s,d): PE broadcasts s = 2b+h (f32r) and h (fp16) to
     128 partitions; Act computes w = |2*tseq - s| (fp16); Vector computes
     v = h - w (fp16, 2x) = 2*tent, then an exact min/max tournament
     produces top-2 (duplicate-safe, all fp16 tensor_tensor at 2x).
  5. lam = max(top,0)*0.5 + tseq (f32); PE transpose; single output DMA.
"""
import contextlib

import numpy as np

import concourse.bass as bass
import concourse.bacc as bacc
import concourse.mybir as mybir
from concourse.tile import TileContext

F32 = mybir.dt.float32
F32R = mybir.dt.float32r
F16 = mybir.dt.float16
I32 = mybir.dt.int32
I16 = mybir.dt.int16
U32 = mybir.dt.uint32
ALU = mybir.AluOpType
AX = mybir.AxisListType
ACT = mybir.ActivationFunctionType

B = 128
N = 65536
P = 1024
T = 512
NCORES = 8
SPC = 16                  # samples per core
NSD = 32                  # (sample, dim) per core
PW = 576                  # per-dim slot block (max dim count 553)
NSLOT = 4 * PW            # 2304 slots/sample: [b0|b1|d0|d1]
NIW = NSLOT // 16         # 144 wrapped idx cols per round
PWL = 320                 # live slots after compaction (max live 292)
SPIN_W = 44               # sparse in cols: 36 data + 8 pads
SPF = PW // 16            # 36
TMS = 36                  # stride between first-2 slot positions
WRAP_COLMAJOR = True      # sparse wrap: logical i <-> [i%16, i//16]


def _install_axon_shim():
    import sys
    import types

    if "antenv.axon_hooks" in sys.modules:
        return
    mod = types.ModuleType("antenv.axon_hooks")
    mod._hook = None
    mod.set_axon_ntff_profile_hook = lambda h: setattr(mod, "_hook", h)
    mod.get_axon_ntff_profile_hook = lambda: mod._hook
    sys.modules["antenv.axon_hooks"] = mod
    try:
        import antenv

        antenv.axon_hooks = mod
    except ImportError:
        pass
    try:
        from trn_agent_boot.trn_boot import _ntff_profile_via_ctypes

        mod._hook = _ntff_profile_via_ctypes("/opt/axon/libaxon_pjrt.so")
    except Exception:
        pass


# fp16 pair [h=0x0000 | b=0x4000(=2.0)] as one little-endian f32 word
PADWORD = float(np.frombuffer(np.uint32(0x00004000).tobytes(), np.float32)[0])


def build_nc():
    nc = bacc.Bacc("TRN2", target_bir_lowering=False, debug=False)
    dtm = nc.dram_tensor("dtm", [SPC, N], F32, kind="ExternalInput")
    locw = nc.dram_tensor("locw", [128, 2 * NIW], I16, kind="ExternalInput")
    maskE_in = nc.dram_tensor("maskE", [SPC, NSLOT], F16, kind="ExternalInput")
    maskO_in = nc.dram_tensor("maskO", [SPC, NSLOT], F16, kind="ExternalInput")
    selR_in = nc.dram_tensor("selR", [128, 1], F32, kind="ExternalInput")
    itc_in = nc.dram_tensor("itc", [128, 4], F32, kind="ExternalInput")
    ones_in = nc.dram_tensor("ones_c", [1, 128], F32, kind="ExternalInput")
    ident_in = nc.dram_tensor("ident", [128, 128], F32, kind="ExternalInput")
    sdram = nc.dram_tensor("sdram", [NSD, PWL], F32, kind="Internal")
    hdram = nc.dram_tensor("hdram", [NSD, PWL], F16, kind="Internal")
    pkdram = nc.dram_tensor("pkdram", [SPC, NSLOT // 2], F32, kind="Internal")
    out = nc.dram_tensor("out", [SPC, 2, T, 2], F32, kind="ExternalOutput")

    with TileContext(nc) as tc:
        with contextlib.ExitStack() as _st:
            gp = _st.enter_context(tc.tile_pool(name="gp", bufs=1))
            id_t = gp.tile([128, 128], F32, tag="ident")
            nc.sync.dma_start(id_t[:], ident_in[:])
            idf16 = gp.tile([128, 128], F16, tag="idf16")
            nc.vector.tensor_copy(idf16[:], id_t[:])
            ones_t = gp.tile([1, 128], F32, tag="ones")
            nc.sync.dma_start(ones_t[:], ones_in[:])
            itc_t = gp.tile([128, 4], F32, tag="itc")
            nc.sync.dma_start(itc_t[:], itc_in[:])
            selR = gp.tile([128, 1], F32, tag="selR")
            nc.sync.dma_start(selR[:], selR_in[:])

            sh_rows = gp.tile([NSD, PWL], F32, tag="sh_rows")
            h_rows = gp.tile([NSD, PWL], F16, tag="h_rows")
            spout = gp.tile([16, NSD * SPIN_W], F32, tag="spout")
            nf_t = gp.tile([1, NSD], U32, tag="nf")
            tseq = gp.tile([128, 128], F32, tag="tseq")
            tseq2 = gp.tile([128, 128], F32, tag="tseq2")
            lam = gp.tile([128, 256], F32, tag="lam")

            # ---------------- gather + merge + pack + tmin ----------------
            _gsid, _ = nc.enter_named_scope("gatherphase", False)
            with tc.tile_pool(name="dtp", bufs=1) as dtp:
                dt16 = dtp.tile([128, N], F16, tag="dt16")
                locw_t = dtp.tile([128, 2 * NIW], I16, tag="locw")
                nc.sync.dma_start(locw_t[:], locw[:])
                maskE = dtp.tile([128, NSLOT], F16, tag="maskE")
                maskO = dtp.tile([128, NSLOT], F16, tag="maskO")
                m_t = dtp.tile([128, 2 * NSLOT], F16, tag="m_t")
                mrg = dtp.tile([128, NSLOT], F16, tag="mrg")
                pkg = dtp.tile([128, 2 * (NSLOT // 2)], F32, tag="pkg")
                # table loads: casting DMAs must go via gpsimd
                for k in range(8):
                    for r in range(2):
                        si = 8 * r + k
                        p0 = 16 * k + r
                        nc.gpsimd.dma_start(dt16[p0:p0 + 1, :],
                                            dtm[si:si + 1, :])
                        nc.sync.dma_start(maskE[p0:p0 + 1, :],
                                          maskE_in[si:si + 1, :])
                        nc.sync.dma_start(maskO[p0:p0 + 1, :],
                                          maskO_in[si:si + 1, :])
                ga = dtp.tile([128, 2 * NSLOT], F16, tag="ga")
                for r in range(2):
                    nc.gpsimd.ap_gather(
                        ga[:], dt16[:], locw_t[:, NIW * r:NIW * (r + 1)],
                        channels=128, num_elems=N // 2, d=2, num_idxs=NSLOT)
                    gav = ga[:].rearrange("p (q e) -> p q e", e=2)
                    msl = slice(NSLOT * r, NSLOT * (r + 1))
                    # m = e*maskE + o*maskO
                    nc.vector.tensor_tensor(out=m_t[:, msl], in0=gav[:, :, 0],
                                            in1=maskE[:], op=ALU.mult)
                    nc.vector.tensor_tensor(out=mrg[:],
                                            in0=gav[:, :, 1],
                                            in1=maskO[:], op=ALU.mult)
                    nc.vector.tensor_tensor(out=m_t[:, msl], in0=m_t[:, msl],
                                            in1=mrg[:], op=ALU.add)
                # pack words [b | h] interleaved as fp16 pairs -> f32 words
                pkv = pkg[:].bitcast(F16).rearrange(
                    "p (r q e) -> p r q e", r=2, e=2)
                mv = m_t[:].rearrange("p (r h q) -> p r h q", r=2, h=2)
                nc.vector.tensor_copy(pkv[:, :, :, 0], mv[:, :, 0, :])
                nc.vector.tensor_tensor(
                    out=pkv[:, :, :, 1],
                    in0=mv[:, :, 1, :],
                    in1=mv[:, :, 0, :],
                    op=ALU.subtract)

                # tmin/tmax from first-2 slots (positions 0 and TMS per block)
                tmn = [None, None]
                tmx = [None, None]
                for r in range(2):
                    base = NSLOT * r
                    bv = m_t[:, base:base + 2 * PW].rearrange(
                        "p (d z) -> p d z", d=2)[:, :, 0:TMS + 1:TMS]
                    dv = m_t[:, base + 2 * PW:base + 4 * PW].rearrange(
                        "p (d z) -> p d z", d=2)[:, :, 0:TMS + 1:TMS]
                    tmn_r = gp.tile([128, 2], F32, tag=f"tmn{r}")
                    tmx_r = gp.tile([128, 2], F32, tag=f"tmx{r}")
                    nc.vector.tensor_reduce(out=tmn_r[:], in_=bv,
                                            axis=AX.X, op=ALU.min)
                    nc.vector.tensor_reduce(out=tmx_r[:], in_=dv,
                                            axis=AX.X, op=ALU.max)
                    tmn[r], tmx[r] = tmn_r, tmx_r
                # per-partition select: partition 16k+r takes round r
                tm4 = gp.tile([128, 4], F32, tag="tm4")
                dmn = gp.tile([128, 2], F32, tag="dmn")
                nc.vector.tensor_tensor(out=dmn[:], in0=tmn[1][:],
                                        in1=tmn[0][:], op=ALU.subtract)
                nc.vector.scalar_tensor_tensor(
                    out=tm4[:, 0:2], in0=dmn[:], scalar=selR[:, 0:1],
                    in1=tmn[0][:], op0=ALU.mult, op1=ALU.add)
                nc.vector.tensor_tensor(out=dmn[:], in0=tmx[1][:],
                                        in1=tmx[0][:], op=ALU.subtract)
                nc.vector.scalar_tensor_tensor(
                    out=tm4[:, 2:4], in0=dmn[:], scalar=selR[:, 0:1],
                    in1=tmx[0][:], op0=ALU.mult, op1=ALU.add)
                # tm4 = [tmin_d0, tmin_d1, tmax_d0, tmax_d1] -> deltas
                nc.vector.tensor_tensor(out=tm4[:, 2:4], in0=tm4[:, 2:4],
                                        in1=tm4[:, 0:2], op=ALU.subtract)
                nc.vector.tensor_scalar(out=tm4[:, 2:4], in0=tm4[:, 2:4],
                                        scalar1=1.0 / 511.0, scalar2=None,
                                        op0=ALU.mult)

                # sparse compaction per (s,d); valid pkg rows bounce via DRAM
                spin = dtp.tile([16, NSD * SPIN_W], F32, tag="spin")
                nc.vector.memset(spin[:], PADWORD)
                for si in range(SPC):
                    r, k = si // 8, si % 8
                    row = 16 * k + r
                    nc.sync.dma_start(
                        pkdram.ap()[si:si + 1, :],
                        pkg[row:row + 1,
                            (NSLOT // 2) * r:(NSLOT // 2) * (r + 1)])
                for sdx in range(NSD):
                    si, d = sdx // 2, sdx % 2
                    src = pkdram.ap()[si:si + 1, d * PW:(d + 1) * PW].rearrange(
                        "o (p f) -> (o p) f", p=16)
                    nc.sync.dma_start(
                        spin[:, sdx * SPIN_W:sdx * SPIN_W + SPF], src)
                for sdx in range(NSD):
                    nc.gpsimd.sparse_gather(
                        spout[:, sdx * SPIN_W:(sdx + 1) * SPIN_W],
                        spin[:, sdx * SPIN_W:(sdx + 1) * SPIN_W],
                        num_found=nf_t[0:1, sdx:sdx + 1])
                # unpack: s = 2*b + h (f32), h (fp16); take first 320 logical
                SPO = PWL // 16  # 20
                sov = spout[:].bitcast(F16).rearrange(
                    "p (s w e) -> p s w e", s=NSD, e=2)
                s2t = gp.tile([16, NSD * SPO], F32, tag="s2t")
                h2t = gp.tile([16, NSD * SPO], F16, tag="h2t")
                nc.vector.tensor_scalar(
                    out=s2t[:].rearrange("p (s w) -> p s w", s=NSD),
                    in0=sov[:, :, 0:SPO, 0], scalar1=2.0, scalar2=None,
                    op0=ALU.mult)
                nc.vector.tensor_tensor(
                    out=s2t[:].rearrange("p (s w) -> p s w", s=NSD),
                    in0=s2t[:].rearrange("p (s w) -> p s w", s=NSD),
                    in1=sov[:, :, 0:SPO, 1], op=ALU.add)
                nc.vector.tensor_copy(
                    h2t[:].rearrange("p (s w) -> p s w", s=NSD),
                    sov[:, :, 0:SPO, 1])
                # unwrap via DRAM bounce: [16, 20] wrapped -> [1, 320] rows
                for sdx in range(NSD):
                    sdst = sdram.ap()[sdx:sdx + 1, :].rearrange(
                        "o (p f) -> (o p) f", p=16)
                    hdst = hdram.ap()[sdx:sdx + 1, :].rearrange(
                        "o (p f) -> (o p) f", p=16)
                    nc.sync.dma_start(sdst,
                                      s2t[:, sdx * SPO:(sdx + 1) * SPO])
                    nc.sync.dma_start(hdst,
                                      h2t[:, sdx * SPO:(sdx + 1) * SPO])
                nc.sync.dma_start(sh_rows[:], sdram[:])
                nc.sync.dma_start(h_rows[:], hdram[:])
            nc.leave_named_scope("gatherphase", _gsid, False)

            # ---------------- tseq ----------------
            with tc.tile_pool(name="tqp", bufs=1, space="PSUM") as tqp:
                tmT_p = tqp.tile([4, 128], F32, tag="tmT")
                nc.tensor.transpose(out=tmT_p[:], in_=tm4[:], identity=id_t[:])
                tmT = gp.tile([4, 128], F32, tag="tmTs")
                nc.vector.tensor_copy(tmT[:], tmT_p[:])
                # rhs cols iterate (r outer, k inner) = sample index si
                rhs4 = tmT[0:4, :].rearrange(
                    "p (k f) -> p f k", k=8)[:, 0:2, :]
                for d in range(2):
                    tmb_p = tqp.tile([128, 16], F32, tag=f"tmb{d}")
                    dlb_p = tqp.tile([128, 16], F32, tag=f"dlb{d}")
                    nc.tensor.matmul(
                        out=tmb_p[:],
                        lhsT=id_t[0:4, d:d + 1].to_broadcast([4, 128]),
                        rhs=rhs4, start=True, stop=True)
                    nc.tensor.matmul(
                        out=dlb_p[:],
                        lhsT=id_t[0:4, 2 + d:3 + d].to_broadcast([4, 128]),
                        rhs=rhs4, start=True, stop=True)
                    # tseq cols (si, d, c) with sdx = 2*si + d
                    ts3 = tseq[:].rearrange(
                        "p (s dd c) -> p s dd c", dd=2, c=4)[:, :, d, :]
                    itc_v = itc_t[:].unsqueeze(1).to_broadcast([128, 16, 4])
                    dl_v = dlb_p[:].unsqueeze(2).to_broadcast([128, 16, 4])
                    tm_v = tmb_p[:].unsqueeze(2).to_broadcast([128, 16, 4])
                    nc.vector.tensor_tensor(out=ts3, in0=itc_v, in1=dl_v,
                                            op=ALU.mult)
                    nc.vector.tensor_tensor(out=ts3, in0=ts3, in1=tm_v,
                                            op=ALU.add)
                nc.vector.tensor_scalar(out=tseq2[:], in0=tseq[:],
                                        scalar1=2.0, scalar2=None,
                                        op0=ALU.mult)

            # ---------------- tents + tournament ----------------
            _tsid, _ = nc.enter_named_scope("tentphase", False)
            GRP = 8
            with tc.tile_pool(name="wp", bufs=3) as wp, \
                 tc.tile_pool(name="tg", bufs=2) as tg, \
                 tc.tile_pool(name="pp", bufs=3, space="PSUM") as pp:
                for g in range(4):
                    tt = tg.tile([128, GRP, 4, 2, PWL // 2], F16, tag="tt")
                    sA = tg.tile([128, GRP, 4, 2, 80], F16, tag="sA")
                    sB = tg.tile([128, GRP, 4, 2, 40], F16, tag="sB")
                    sC = tg.tile([128, GRP, 4, 2, 20], F16, tag="sC")
                    sD = tg.tile([128, GRP, 4, 2, 10], F16, tag="sD")
                    sE = tg.tile([128, GRP, 4, 2, 5], F16, tag="sE")
                    sF = tg.tile([128, GRP, 4, 2, 2], F16, tag="sF")
                    sFt = tg.tile([128, GRP, 4, 2], F16, tag="sFt")
                    sG = tg.tile([128, GRP, 4, 2], F16, tag="sG")
                    sGt = tg.tile([128, GRP, 4], F16, tag="sGt")
                    for j in range(GRP):
                        sdx = g * GRP + j
                        sb_p = pp.tile([128, PWL], F32, tag="sb")
                        hb_p = pp.tile([128, PWL], F32, tag="hb")
                        nc.tensor.matmul(
                            out=sb_p[:],
                            lhsT=id_t[0:NSD, sdx:sdx + 1].to_broadcast(
                                [NSD, 128]),
                            rhs=sh_rows[:, :],
                            start=True, stop=True)
                        nc.tensor.matmul(
                            out=hb_p[:],
                            lhsT=idf16[0:NSD, sdx:sdx + 1].to_broadcast(
                                [NSD, 128]),
                            rhs=h_rows[:, :], start=True, stop=True)
                        hb = wp.tile([128, PWL], F16, tag="hb16")
                        nc.scalar.activation(hb[:], hb_p[:], ACT.Copy)
                        wv = wp.tile([128, 4, PWL], F16, tag="wv")
                        for c in range(4):
                            nc.scalar.activation(
                                wv[:, c, :], sb_p[:], ACT.Abs,
                                bias=tseq2[:, 4 * sdx + c:4 * sdx + c + 1],
                                scale=-1.0)
                        vt = wp.tile([128, 4, PWL], F16, tag="vt")
                        hb_v = hb[:].unsqueeze(1).to_broadcast([128, 4, PWL])
                        nc.vector.tensor_tensor(out=vt[:], in0=hb_v,
                                                in1=wv[:], op=ALU.subtract)
                        # tournament level 0
                        nc.vector.tensor_tensor(
                            out=tt[:, j, :, 0, :], in0=vt[:, :, 0:160],
                            in1=vt[:, :, 160:320], op=ALU.max)
                        nc.vector.tensor_tensor(
                            out=tt[:, j, :, 1, :], in0=vt[:, :, 0:160],
                            in1=vt[:, :, 160:320], op=ALU.min)
                    # batched merge levels: 160->80->40->20->10->5
                    src_t = tt
                    for dst_t, L in ((sA, 160), (sB, 80), (sC, 40),
                                     (sD, 20), (sE, 10)):
                        h0 = L // 2
                        a1 = src_t[:, :, :, 0, 0:h0]
                        b1 = src_t[:, :, :, 0, h0:L]
                        a2 = src_t[:, :, :, 1, 0:h0]
                        b2 = src_t[:, :, :, 1, h0:L]
                        nc.vector.tensor_tensor(out=dst_t[:, :, :, 0, :],
                                                in0=a1, in1=b1, op=ALU.max)
                        nc.vector.tensor_tensor(out=dst_t[:, :, :, 1, :],
                                                in0=a2, in1=b2, op=ALU.max)
                        nc.vector.tensor_tensor(out=src_t[:, :, :, 0, 0:h0],
                                                in0=a1, in1=b1, op=ALU.min)
                        nc.vector.tensor_tensor(
                            out=dst_t[:, :, :, 1, :],
                            in0=src_t[:, :, :, 0, 0:h0],
                            in1=dst_t[:, :, :, 1, :], op=ALU.max)
                        src_t = dst_t
                    # tail: 5 pairs -> (0,1),(2,3) -> merge -> with 4
                    x1 = sE[:, :, :, 0, :]
                    x2 = sE[:, :, :, 1, :]
                    nc.vector.tensor_tensor(out=sF[:, :, :, 0, :],
                                            in0=x1[:, :, :, 0:3:2],
                                            in1=x1[:, :, :, 1:4:2], op=ALU.max)
                    nc.vector.tensor_tensor(out=sF[:, :, :, 1, :],
                                            in0=x2[:, :, :, 0:3:2],
                                            in1=x2[:, :, :, 1:4:2], op=ALU.max)
                    nc.vector.tensor_tensor(out=x1[:, :, :, 0:2],
                                            in0=x1[:, :, :, 0:3:2],
                                            in1=x1[:, :, :, 1:4:2], op=ALU.min)
                    nc.vector.tensor_tensor(out=sF[:, :, :, 1, :],
                                            in0=x1[:, :, :, 0:2],
                                            in1=sF[:, :, :, 1, :], op=ALU.max)
                    # merge the 2 pairs in sF
                    nc.vector.tensor_tensor(out=sG[:, :, :, 0],
                                            in0=sF[:, :, :, 0, 0],
                                            in1=sF[:, :, :, 0, 1], op=ALU.max)
                    nc.vector.tensor_tensor(out=sFt[:, :, :, 0],
                                            in0=sF[:, :, :, 0, 0],
                                            in1=sF[:, :, :, 0, 1], op=ALU.min)
                    nc.vector.tensor_tensor(out=sG[:, :, :, 1],
                                            in0=sF[:, :, :, 1, 0],
                                            in1=sF[:, :, :, 1, 1], op=ALU.max)
                    nc.vector.tensor_tensor(out=sG[:, :, :, 1],
                                            in0=sFt[:, :, :, 0],
                                            in1=sG[:, :, :, 1], op=ALU.max)
                    # merge with leftover pair 4 -> write lam (f32)
                    lam4 = lam[:].rearrange("p (s c e) -> p s c e", s=32, e=2)
                    lsl = lam4[:, g * GRP:(g + 1) * GRP, :, :]
                    nc.vector.tensor_tensor(out=lsl[:, :, :, 0],
                                            in0=sG[:, :, :, 0],
                                            in1=x1[:, :, :, 4], op=ALU.max)
                    nc.vector.tensor_tensor(out=sGt[:],
                                            in0=sG[:, :, :, 0],
                                            in1=x1[:, :, :, 4], op=ALU.min)
                    nc.vector.tensor_tensor(out=sG[:, :, :, 1],
                                            in0=sG[:, :, :, 1],
                                            in1=x2[:, :, :, 4], op=ALU.max)
                    nc.vector.tensor_tensor(out=lsl[:, :, :, 1],
                                            in0=sGt[:],
                                            in1=sG[:, :, :, 1], op=ALU.max)
            nc.leave_named_scope("tentphase", _tsid, False)

            # ---------------- finalize + output ----------------
            nc.vector.tensor_scalar(out=lam[:], in0=lam[:], scalar1=0.0,
                                    scalar2=0.5, op0=ALU.max, op1=ALU.mult)
            lam3 = lam[:].rearrange("p (s e) -> p s e", e=2)
            tseq_v = tseq[:].unsqueeze(2).to_broadcast([128, 128, 2])
            nc.vector.tensor_tensor(out=lam3, in0=lam3, in1=tseq_v,
                                    op=ALU.add)
            outT = gp.tile([128, 256], F32, tag="outT")
            with tc.tile_pool(name="op", bufs=1, space="PSUM") as op:
                for kk in range(2):
                    lT = op.tile([128, 128], F32, tag=f"lT{kk}")
                    nc.tensor.transpose(
                        out=lT[:],
                        in_=lam[:].rearrange("p (s e) -> p s e", e=2)[:, :, kk],
                        identity=id_t[:])
                    nc.vector.tensor_copy(
                        outT[:].rearrange("p (s e) -> p s e", e=2)[:, :, kk],
                        lT[:])
            dst = out.ap().rearrange(
                "s d (c t) e -> (s d c) (t e)", c=4)
            nc.sync.dma_start(dst, outT[:])
    nc.compile()
    return nc


_NC_CACHE = None
_LAST_IN_MAPS = None


def _host_prep(dtm_val, birth_loc, death_loc, ph_dim):
    """Build per-core input maps (index/layout data only)."""
    itc = np.zeros((128, 4), np.float32)
    for c in range(4):
        itc[:, c] = 128 * c + np.arange(128)
    ones_c = np.ones((1, 128), np.float32)
    ident = np.eye(128, dtype=np.float32)
    selR = np.zeros((128, 1), np.float32)
    for k in range(8):
        selR[16 * k + 1, 0] = 1.0

    jj = np.arange(PW)
    if WRAP_COLMAJOR:
        posn = SPF * (jj % 16) + jj // 16
    else:
        posn = jj.copy()

    in_maps = []
    for i in range(NCORES):
        s0 = i * SPC
        locc = np.zeros((SPC, NSLOT), np.int32)
        mE = np.zeros((SPC, NSLOT), np.float16)
        mO = np.zeros((SPC, NSLOT), np.float16)
        for si in range(SPC):
            s = s0 + si
            v0 = np.float16(dtm_val[s, 0])
            v1 = np.float16(dtm_val[s, 1])
            assert float(v0) + float(v1) > 0.0, "pad-death trick needs v0+v1>0"
            for d in range(2):
                pos = np.where(ph_dim[s] == d)[0]
                n = len(pos)
                assert 2 <= n <= PW, f"dim count {n} out of range"
                q = posn[:n]
                bl = birth_loc[s, pos]
                dl = death_loc[s, pos]
                locc[si, d * PW + q] = bl
                locc[si, 2 * PW + d * PW + q] = dl
                mE[si, d * PW + q] = (1 - (bl & 1)).astype(np.float16)
                mO[si, d * PW + q] = (bl & 1).astype(np.float16)
                mE[si, 2 * PW + d * PW + q] = (1 - (dl & 1)).astype(np.float16)
                mO[si, 2 * PW + d * PW + q] = (dl & 1).astype(np.float16)
                # pad slots: birth = +(e+o), death = -(e+o) -> h < 0, dropped
                qp = posn[n:]
                mE[si, d * PW + qp] = 1.0
                mO[si, d * PW + qp] = 1.0
                mE[si, 2 * PW + d * PW + qp] = -1.0
                mO[si, 2 * PW + d * PW + qp] = -1.0
        # wrapped int16 gather indices, per DSP core and round
        locw = np.zeros((128, 2 * NIW), np.int16)
        for k in range(8):
            for r in range(2):
                si = 8 * r + k
                lst = (locc[si] >> 1).astype(np.int16)
                locw[16 * k:16 * (k + 1), NIW * r:NIW * (r + 1)] = (
                    lst.reshape(NIW, 16).T)
        # dtm rows reordered so row si maps to table partition 16k+r
        in_maps.append({
            "dtm": np.ascontiguousarray(dtm_val[s0:s0 + SPC]),
            "locw": locw,
            "maskE": mE,
            "maskO": mO,
            "selR": selR,
            "itc": itc,
            "ones_c": ones_c,
            "ident": ident,
        })
    return in_maps


def kernel(dtm_val, birth_loc, death_loc, ph_dim):
    global _NC_CACHE, _LAST_IN_MAPS
    _install_axon_shim()
    from concourse.bass_utils import run_bass_kernel_spmd

    dtm_val = np.ascontiguousarray(np.asarray(dtm_val, dtype=np.float32))
    birth_loc = np.asarray(birth_loc, dtype=np.int32)
    death_loc = np.asarray(death_loc, dtype=np.int32)
    ph_dim = np.asarray(ph_dim, dtype=np.int32)

    if _NC_CACHE is None:
        _NC_CACHE = build_nc()
    nc = _NC_CACHE

    in_maps = _host_prep(dtm_val, birth_loc, death_loc, ph_dim)
    _LAST_IN_MAPS = in_maps
    res = run_bass_kernel_spmd(nc, in_maps, core_ids=list(range(NCORES)))
    outs = [r["out"] for r in res.results]
    return np.concatenate(outs, axis=0).astype(np.float32)
